# revision 9
# baseline (speedup 1.0000x reference)
"""Trainium2 Bass kernel: sparse AE encoder (L1 fan-in-1 -> relu/BN -> L2 block-diag
4x4 -> relu/BN -> L3 sparse 256-nnz/TF -> BN), SPMD over 8 NeuronCores.

Sharding: gene/hidden axis across cores (BN1/BN2 local: every core holds all 256
batch rows of its features). All layers are TensorEngine matmuls with host-packed
stationaries (L1 scatter matrix, L2 block-diagonal, L3 densified W3 shard in bf16).
Partial z is computed in two TF-halves; each half is transposed on PE, AllToAll'd
(the first overlaps the second half's matmuls), reduced in fp32 on-core, and BN3'd
per 64-TF chunk. Each core emits a [128, 256] outT shard (two 64-row chunks).
"""

import numpy as np
import ml_dtypes

import concourse.bacc as bacc
import concourse.bass as bass
import concourse.tile as tile
import concourse.mybir as mybir
from concourse import bass_utils
from concourse.masks import make_identity

N_GENES = 8192
WM = 4
HID = N_GENES * WM          # 32768
N_TF = 1024
B = 256
EPS = 1e-5

NCORES = 8
GSH = N_GENES // NCORES     # 1024 genes / core
HSH = HID // NCORES         # 4096 hidden rows / core
P = 128
NT = HSH // P               # 32 hidden tiles / core
NGT = GSH // P              # 8 gene tiles / core
GB = 8                      # stats batching group size (tiles)
TH = 512                    # TF half width
CH = 64                     # TF rows per core per half (A2A shard)

BF16 = ml_dtypes.bfloat16
F32 = mybir.dt.float32
BF = mybir.dt.bfloat16
AF = mybir.ActivationFunctionType
OP = mybir.AluOpType

TRACE = False
LAST_RESULT = None

_cache = {}


def _build_graph():
    nc = bacc.Bacc("TRN2", target_bir_lowering=False, debug=False, num_devices=NCORES)

    xd = nc.dram_tensor("xd", [P, NGT * B], BF, kind="ExternalInput").ap()
    e1d = nc.dram_tensor("e1d", [P, NT * P], BF, kind="ExternalInput").ap()
    w2d = nc.dram_tensor("w2d", [P, NT * P], BF, kind="ExternalInput").ap()
    w3d = nc.dram_tensor("w3d", [P, NT * N_TF], BF, kind="ExternalInput").ap()
    b1d = nc.dram_tensor("b1d", [P, NT], F32, kind="ExternalInput").ap()
    b2d = nc.dram_tensor("b2d", [P, NT], F32, kind="ExternalInput").ap()
    outT = nc.dram_tensor("outT", [P, B], F32, kind="ExternalOutput").ap()

    from contextlib import ExitStack
    with tile.TileContext(nc) as tc, ExitStack() as ctx:
        cpool = ctx.enter_context(tc.tile_pool(name="const", bufs=1))
        wpool = ctx.enter_context(tc.tile_pool(name="wts", bufs=1))
        apool = ctx.enter_context(tc.tile_pool(name="acts", bufs=1))
        spool = ctx.enter_context(tc.tile_pool(name="stats", bufs=1))
        ztpool = ctx.enter_context(tc.tile_pool(name="ztile", bufs=3))
        psAB = ctx.enter_context(tc.tile_pool(name="psAB", bufs=2, space="PSUM"))
        psZp = ctx.enter_context(tc.tile_pool(name="psZ", bufs=1, space="PSUM"))
        psTp = ctx.enter_context(tc.tile_pool(name="psT", bufs=2, space="PSUM"))
        dpool = ctx.enter_context(tc.tile_pool(name="dram", bufs=1, space="DRAM"))

        # ---- static loads (contiguous, few instructions) -----------------
        xs = wpool.tile([P, NGT * B], BF, name="xs")
        nc.sync.dma_start(xs[:], xd[:])
        b1t = cpool.tile([P, NT], F32, name="b1t")
        nc.sync.dma_start(b1t[:], b1d[:])
        e1s = wpool.tile([P, NT * P], BF, name="e1s")
        nc.sync.dma_start(e1s[:], e1d[:])
        b2t = cpool.tile([P, NT], F32, name="b2t")
        nc.sync.dma_start(b2t[:], b2d[:])
        w2s = wpool.tile([P, NT * P], BF, name="w2s")
        nc.sync.dma_start(w2s[:], w2d[:])
        w3s = wpool.tile([P, NT * N_TF], BF, name="w3s")
        W3CH = 8
        cw = NT * N_TF // W3CH
        for c in range(W3CH):
            nc.sync.dma_start(w3s[:, c * cw:(c + 1) * cw], w3d[:, c * cw:(c + 1) * cw])

        idt = cpool.tile([P, P], BF, name="idt")
        make_identity(nc, idt[:])
        epst = cpool.tile([P, 1], F32, name="epst")
        nc.gpsimd.memset(epst[:], EPS)

        hr = apool.tile([P, NT * B], BF, name="hr")
        h1n = apool.tile([P, NT * B], BF, name="h1n")
        h2n = apool.tile([P, NT * B], BF, name="h2n")

        def batched_norm_params(st, g0, istd, nm):
            """From bn_stats 6-tuples (even/odd halves) of GB tiles, produce
            istd and -mean*istd, batched on [P, GB]."""
            sv = st[:, g0 * 6:(g0 + GB) * 6].rearrange("p (t s) -> p t s", s=6)
            me, mo = sv[:, :, 1], sv[:, :, 4]
            M2e, M2o = sv[:, :, 2], sv[:, :, 5]
            ms = spool.tile([P, GB], F32, name="ms", tag="ms")
            nc.vector.scalar_tensor_tensor(ms[:], in0=me, scalar=1.0, in1=mo,
                                           op0=OP.mult, op1=OP.add)
            dm = spool.tile([P, GB], F32, name="dm", tag="dm")
            nc.vector.scalar_tensor_tensor(dm[:], in0=me, scalar=1.0, in1=mo,
                                           op0=OP.mult, op1=OP.subtract)
            vq = spool.tile([P, GB], F32, name="vq", tag="vq")
            nc.vector.scalar_tensor_tensor(vq[:], in0=dm[:], scalar=0.25, in1=dm[:],
                                           op0=OP.mult, op1=OP.mult)
            var = spool.tile([P, GB], F32, name="var", tag="var")
            nc.vector.scalar_tensor_tensor(var[:], in0=M2e[:, :], scalar=1.0,
                                           in1=M2o[:, :], op0=OP.mult, op1=OP.add)
            nc.vector.tensor_scalar(out=var[:], in0=var[:], scalar1=1.0 / B,
                                    scalar2=None, op0=OP.mult)
            nc.vector.tensor_tensor(var[:], var[:], vq[:], op=OP.add)
            std = spool.tile([P, GB], F32, name="std", tag="std")
            nc.scalar.activation(std[:], var[:], AF.Sqrt, bias=epst[:, 0:1])
            nc.vector.reciprocal(istd[:], std[:])
            nc.vector.scalar_tensor_tensor(nm[:], in0=ms[:], scalar=-0.5, in1=istd[:],
                                           op0=OP.mult, op1=OP.mult)

        def phase(lhs_s, rhs_get, btile, dst, hrbuf):
            """sparse-linear + relu + batchnorm -> bf16 dst; stats per GB-tile group."""
            st = spool.tile([P, NT * 6], F32, name="st", tag="st")
            for g0 in range(0, NT, GB):
                for t in range(g0, g0 + GB):
                    ps = psAB.tile([P, B], F32, name="psL", tag="psL")
                    nc.tensor.matmul(ps[:], lhsT=lhs_s[:, t * P:(t + 1) * P],
                                     rhs=rhs_get(t), start=True, stop=True)
                    hrt = hrbuf[:, t * B:(t + 1) * B]
                    nc.scalar.activation(hrt, ps[:], AF.Relu, bias=btile[:, t:t + 1])
                for u in range(g0, g0 + GB):
                    nc.vector.bn_stats(st[:, u * 6:(u + 1) * 6],
                                       hrbuf[:, u * B:(u + 1) * B])
                istd = spool.tile([P, GB], F32, name="istd", tag="istd")
                nm = spool.tile([P, GB], F32, name="nm", tag="nm")
                batched_norm_params(st, g0, istd, nm)
                for t in range(g0, g0 + GB):
                    eng = nc.vector if (t % 8) < 5 else nc.gpsimd
                    eng.tensor_scalar(out=dst[:, t * B:(t + 1) * B],
                                      in0=hrbuf[:, t * B:(t + 1) * B],
                                      scalar1=istd[:, t - g0:t - g0 + 1],
                                      scalar2=nm[:, t - g0:t - g0 + 1],
                                      op0=OP.mult, op1=OP.add)

        phase(e1s, lambda t: xs[:, (t // 4) * B:(t // 4 + 1) * B], b1t, h1n, hr)
        phase(w2s, lambda t: h1n[:, t * B:(t + 1) * B], b2t, h2n, hr)

        # ---- layer 3 in two TF-halves; A2A of half 0 overlaps half 1 -----
        psZ = [[psZp.tile([P, TH], F32, name=f"psZ{bh}{th}", tag=f"psZ{bh}{th}")
                for th in range(2)] for bh in range(2)]
        za = [dpool.tile([NCORES * CH, B], BF, name=f"za{th}") for th in range(2)]

        for th in range(2):
            for t in range(NT):
                for bh in range(2):
                    nc.tensor.matmul(
                        psZ[bh][th][:],
                        lhsT=h2n[:, t * B + bh * P: t * B + (bh + 1) * P],
                        rhs=w3s[:, t * N_TF + th * TH: t * N_TF + (th + 1) * TH],
                        start=(t == 0), stop=(t == NT - 1))
            zpart = apool.tile([P, 2 * TH], BF, name=f"zpart{th}")
            for bh in range(2):
                nc.vector.tensor_copy(zpart[:, bh * TH:(bh + 1) * TH], psZ[bh][th][:])
            zinT = dpool.tile([TH, B], BF, name=f"zinT{th}")
            for tt in range(TH // P):
                zTs = ztpool.tile([P, B], BF, name="zTs", tag="zTs")
                for bh in range(2):
                    pst = psTp.tile([P, P], BF, name="pst", tag="pst")
                    nc.tensor.transpose(
                        pst[:], in_=zpart[:, bh * TH + tt * P: bh * TH + (tt + 1) * P],
                        identity=idt[:])
                    nc.vector.tensor_copy(zTs[:, bh * P:(bh + 1) * P], pst[:])
                nc.gpsimd.dma_start(zinT[tt * P:(tt + 1) * P, :], zTs[:])
            nc.gpsimd.collective_compute(
                "AllToAll", OP.bypass,
                replica_groups=[list(range(NCORES))],
                ins=[zinT.opt()], outs=[za[th].opt()])

        # ---- per-half: load 8 partial slices, tree-reduce fp32, BN3 ------
        for th in range(2):
            zsl8 = ztpool.tile([CH, NCORES * B], BF, name="zsl8", tag="zsl8")
            nc.gpsimd.dma_start(
                zsl8[:].rearrange("p (j b) -> p j b", j=NCORES),
                za[th][:].rearrange("(j p) b -> p j b", p=CH))
            r4 = ztpool.tile([CH, 4 * B], F32, name="r4", tag="r4")
            nc.vector.tensor_tensor(r4[:], zsl8[:, :4 * B], zsl8[:, 4 * B:], op=OP.add)
            r2 = ztpool.tile([CH, 2 * B], F32, name="r2", tag="r2")
            nc.vector.tensor_tensor(r2[:], r4[:, :2 * B], r4[:, 2 * B:], op=OP.add)
            zs = ztpool.tile([CH, B], F32, name="zs", tag="zs")
            nc.vector.tensor_tensor(zs[:], r2[:, :B], r2[:, B:], op=OP.add)

            st6 = spool.tile([CH, 6], F32, name="st6", tag="st6")
            nc.vector.bn_stats(st6[:], zs[:])
            mv3 = spool.tile([CH, 2], F32, name="mv3", tag="mv3")
            nc.vector.bn_aggr(mv3[:], st6[:])
            std3 = spool.tile([CH, 1], F32, name="std3", tag="std3")
            nc.scalar.activation(std3[:], mv3[:, 1:2], AF.Sqrt, bias=epst[:CH, 0:1])
            istd3 = spool.tile([CH, 1], F32, name="istd3", tag="istd3")
            nc.vector.reciprocal(istd3[:], std3[:])
            nm3 = spool.tile([CH, 1], F32, name="nm3", tag="nm3")
            nc.vector.scalar_tensor_tensor(nm3[:], in0=mv3[:, 0:1], scalar=-1.0,
                                           in1=istd3[:], op0=OP.mult, op1=OP.mult)
            ofin = ztpool.tile([CH, B], F32, name="ofin", tag="ofin")
            nc.vector.tensor_scalar(out=ofin[:], in0=zs[:], scalar1=istd3[:],
                                    scalar2=nm3[:], op0=OP.mult, op1=OP.add)
            nc.sync.dma_start(outT[th * CH:(th + 1) * CH, :], ofin[:])

    nc.compile()
    return nc


def _pack_inputs(features, w1, b1, w2, b2, w3, b3,
                 rows1, cols1, rows2, cols2, rows3, cols3):
    """Host-side packing into per-core contiguous [128, N] tile layouts."""
    f32 = np.float32
    features = np.asarray(features, f32)
    w1 = np.asarray(w1, f32); b1 = np.asarray(b1, f32)
    w2 = np.asarray(w2, f32); b2 = np.asarray(b2, f32)
    w3 = np.asarray(w3, f32)
    rows1 = np.asarray(rows1); cols1 = np.asarray(cols1)
    rows2 = np.asarray(rows2); cols2 = np.asarray(cols2)
    rows3 = np.asarray(rows3); cols3 = np.asarray(cols3)

    w1r = np.empty(HID, f32); w1r[rows1] = w1
    c1r = np.empty(HID, np.int64); c1r[rows1] = cols1

    order2 = np.argsort(rows2, kind="stable")
    r2 = rows2[order2]; c2 = cols2[order2]; v2 = w2[order2]

    W3d = np.zeros((HID, N_TF), f32)
    np.add.at(W3d, (cols3.astype(np.int64), rows3.astype(np.int64)), w3)

    in_maps = []
    for c in range(NCORES):
        hbase = c * HSH
        gbase = c * GSH
        xd = np.ascontiguousarray(
            features[:, gbase:gbase + GSH].T.reshape(NGT, P, B).transpose(1, 0, 2)
            .reshape(P, NGT * B)).astype(BF16)

        e1 = np.zeros((NT, P, P), f32)
        w2t = np.zeros((NT, P, P), f32)
        for t in range(NT):
            R0 = hbase + t * P
            G0 = gbase + (t // 4) * P
            rows = np.arange(R0, R0 + P)
            e1[t][c1r[rows] - G0, np.arange(P)] = w1r[rows]
            es = slice(WM * R0, WM * (R0 + P))
            np.add.at(w2t[t], (c2[es] - R0, r2[es] - R0), v2[es])

        w3t = W3d[hbase:hbase + HSH].reshape(NT, P, N_TF)

        in_maps.append({
            "xd": xd,
            "e1d": np.ascontiguousarray(e1.transpose(1, 0, 2).reshape(P, NT * P)).astype(BF16),
            "w2d": np.ascontiguousarray(w2t.transpose(1, 0, 2).reshape(P, NT * P)).astype(BF16),
            "w3d": np.ascontiguousarray(w3t.transpose(1, 0, 2).reshape(P, NT * N_TF)).astype(BF16),
            "b1d": np.ascontiguousarray(b1[hbase:hbase + HSH].reshape(NT, P).T),
            "b2d": np.ascontiguousarray(b2[hbase:hbase + HSH].reshape(NT, P).T),
        })
    return in_maps


def kernel(**inputs) -> np.ndarray:
    global LAST_RESULT
    if "nc" not in _cache:
        _cache["nc"] = _build_graph()
    nc = _cache["nc"]

    in_maps = _pack_inputs(**inputs)
    # b3 is dropped: BN3 subtracts the per-TF batch mean, so a per-TF constant
    # bias cancels exactly.

    res = bass_utils.run_bass_kernel_spmd(
        nc, in_maps, core_ids=list(range(NCORES)), trace=TRACE)
    LAST_RESULT = res

    # core c's outT rows: [0:64] -> TFs [64c, 64c+64); [64:128] -> TFs [512+64c, ...)
    out_T = np.zeros((N_TF, B), np.float32)
    for c in range(NCORES):
        o = res.results[c]["outT"]
        out_T[CH * c: CH * (c + 1)] = o[:CH]
        out_T[TH + CH * c: TH + CH * (c + 1)] = o[CH:]
    return np.ascontiguousarray(out_T.T)
